# revision 1
# baseline (speedup 1.0000x reference)
"""Trainium2 Bass kernel for nn_AttnMLP: 4x (LayerNorm -> Linear(2048,2048) -> tanh-GELU).

Sharding: data-parallel, batch dim (8 batch elements) across 8 NeuronCores.
Weights (4 x 2048 x 2048) replicated per core, held resident in SBUF one
layer at a time.

Per-core dataflow (token-major layout [tokens, features]):
  for layer l:                      # W_l resident in SBUF (16 MB)
    for token tile i (16 x 128 tokens), software-pipelined one tile ahead
      prep(i): DMA x tile [128, 2048]; LN via bn_stats/bn_aggr + in-place
        normalize (DVE); PE-transpose x_norm into 16 [128,128] chunks
        (via PSUM, DVE evacuates to an SBUF xT tile)
      k-outer matmul: PSUM[t=128, e=512] += xT_k.T @ WT_k[:, e] (fp32r,
        1 cyc/row at N=512; lhsT reused across the 4 e-chunks so walrus'
        LDW elision kicks in)
      epilogue: DVE adds the bias broadcast (built once per layer via a
        K=1 ones x bias matmul), ScalarE applies Gelu_apprx_tanh in place,
        DMA y tile to DRAM (input of next layer)

prep(i+1) is emitted before matmuls(i) so the PE fills its wait-for-copies
gap with the next tile's transposes. Tiny "probe" transposes at each layer
start absorb the weight-DMA semaphore waits early in the PE stream;
_split_matmul_waits hoists any remaining multi-wait instructions onto
standalone EventSemaphores (walrus LW-struct/DMA structs accept one wait).

LN affine (ln_w, ln_b) is folded into W and b on the host:
  W' = W * ln_w[None, :],  b' = b + W @ ln_b

Measured (8-core, axon): ~1.0-1.1 ms steady-state per 4-layer pass
(PE busy floor ~0.95 ms), max rel err 3.7e-4 vs the fp32 reference.
"""

import sys

sys.path.insert(0, "/opt/trn_rl_repo")

import numpy as np

N_LAYERS = 4
D = 2048  # embedding dim
B = 8  # batch (one element per core)
S = 2048  # sequence length
T = S  # tokens per core
P = 128  # partitions
KC = D // P  # 16 contraction chunks
EC = 4  # output-feature chunks
EW = D // EC  # 512 output features per chunk
LN_EPS = 1e-5


def build(nc, T_tokens=T, n_layers=N_LAYERS, use_f32r=True, repeat=1,
          static_w=False):
    """Emit the kernel IR into `nc`. Returns None; tensors are declared here.

    repeat > 1 re-runs the whole n_layers stack that many times (reusing the
    same weights) — a timing-only amplifier for slope measurements."""
    import concourse.bass as bass
    import concourse.mybir as mybir
    import concourse.tile as tile
    from contextlib import ExitStack
    from concourse.masks import make_identity
    from concourse.tile import add_dep_helper

    f32 = mybir.dt.float32
    f32r = mybir.dt.float32r if use_f32r else mybir.dt.float32
    NT = T_tokens // P  # token tiles

    x_d = nc.dram_tensor("x", [T_tokens, D], f32, kind="ExternalInput")
    wt_d = nc.dram_tensor("wt", [n_layers, D, D], f32r, kind="ExternalInput")
    b_d = nc.dram_tensor("b", [n_layers, D], f32r, kind="ExternalInput")
    y_d = nc.dram_tensor("y", [T_tokens, D], f32, kind="ExternalOutput")
    buf0 = nc.dram_tensor("xbuf0", [T_tokens, D], f32)
    buf1 = nc.dram_tensor("xbuf1", [T_tokens, D], f32)

    n_steps = n_layers * repeat
    chain = [x_d] + [buf0, buf1] * ((n_steps + 1) // 2)
    srcs = chain[:n_steps]
    dsts = chain[1 : n_steps + 1]
    dsts[-1] = y_d

    wt_v = wt_d.rearrange("l (kc p) e -> l kc p e", p=P)  # [L, 16, 128, 2048]

    with tile.TileContext(nc) as tc, ExitStack() as ctx:
        singles = ctx.enter_context(tc.tile_pool(name="singles", bufs=1))
        wt_pool = ctx.enter_context(tc.tile_pool(name="wt", bufs=1))
        bias_pool = ctx.enter_context(tc.tile_pool(name="bias", bufs=1))
        brep_pool = ctx.enter_context(tc.tile_pool(name="brep", bufs=1))
        x_pool = ctx.enter_context(tc.tile_pool(name="x", bufs=3))
        xt_pool = ctx.enter_context(tc.tile_pool(name="xt", bufs=2))
        y_pool = ctx.enter_context(tc.tile_pool(name="y", bufs=2))
        st_pool = ctx.enter_context(tc.tile_pool(name="st", bufs=4))
        pt_psum = ctx.enter_context(tc.tile_pool(name="ptp", bufs=2, space="PSUM"))
        acc_psum = ctx.enter_context(tc.tile_pool(name="accp", bufs=5, space="PSUM"))
        probe_psum = ctx.enter_context(
            tc.tile_pool(name="probep", bufs=1, space="PSUM")
        )

        ident = singles.tile([P, P], f32)
        make_identity(nc, ident)
        ones_f = singles.tile([1, P], f32)
        nc.vector.memset(ones_f, 1.0)
        ones = singles.tile([1, P], f32r)
        nc.vector.tensor_copy(ones, ones_f)
        eps_t = singles.tile([P, 1], f32)
        nc.vector.memset(eps_t, LN_EPS)

        pending_xT = None
        for step in range(n_steps):
            l = step % n_layers
            src = srcs[step].rearrange("(n p) d -> n p d", p=P)
            dst = dsts[step].rearrange("(n p) d -> n p d", p=P)

            if static_w and step > 0:
                # timing diagnostic (n_layers=1 only): weights stay resident
                wts, bias, brep, wt_probes = static_cache
            else:
                wts = []
                for k in range(KC):
                    w = wt_pool.tile([P, D], f32r, tag=f"wt{k}")
                    nc.sync.dma_start(out=w, in_=wt_v[l, k])
                    wts.append(w)
                bias = bias_pool.tile([1, D], f32r, tag="bias")
                nc.sync.dma_start(out=bias, in_=b_d[l].unsqueeze(0))

                # "Probe" transposes: tiny PE instructions that absorb the
                # DMA waits early in the PE stream, so the fp32r matmuls
                # below carry at most one sync wait and stay back-to-back.
                wt_probes = []
                for k in range(KC):
                    pp = probe_psum.tile(
                        [32, 32], f32, tag="probe", name="probe"
                    )
                    pr = nc.tensor.matmul(
                        out=pp,
                        lhsT=wts[k].bitcast(f32)[0:32, 0:32],
                        rhs=ident[0:32, 0:32],
                        is_transpose=True,
                    )
                    wt_probes.append(pr.ins)
                # bias broadcast [128, D]: one K=1 matmul per 512-chunk, then
                # evacuate to SBUF. Used by the per-tile DVE bias-add.
                brep = brep_pool.tile([P, D], f32, tag="brep")
                for e in range(EC):
                    bacc = acc_psum.tile([P, EW], f32, tag="acc", name="bacc")
                    nc.tensor.matmul(
                        out=bacc,
                        lhsT=ones,
                        rhs=bias[:, bass.ts(e, EW)],
                        start=True,
                        stop=True,
                    )
                    nc.vector.tensor_copy(brep[:, bass.ts(e, EW)], bacc)
                static_cache = (wts, bias, brep, wt_probes)

            def prep(src_view, i):
                """DMA + LayerNorm + PE-transpose for token tile i: returns
                the ready-to-contract xT tile."""
                xt = x_pool.tile([P, D], f32, tag="x", name="xt")
                nc.sync.dma_start(out=xt, in_=src_view[i])

                stats = st_pool.tile([P, 4, 6], f32, tag="bnst", name="stats")
                for g in range(4):
                    nc.vector.bn_stats(
                        out=stats[:, g, :], in_=xt[:, bass.ts(g, 512)]
                    )
                mv = st_pool.tile([P, 2], f32, tag="mv", name="mv")
                nc.vector.bn_aggr(out=mv, in_=stats)
                rstd = st_pool.tile([P, 1], f32, tag="rstd", name="rstd")
                nc.scalar.activation(
                    out=rstd,
                    in_=mv[:, 1:2],
                    func=mybir.ActivationFunctionType.Sqrt,
                    bias=eps_t,
                    scale=1.0,
                )
                nc.vector.reciprocal(out=rstd, in_=rstd)
                nc.vector.tensor_scalar(
                    out=xt,
                    in0=xt,
                    scalar1=mv[:, 0:1],
                    scalar2=rstd,
                    op0=mybir.AluOpType.subtract,
                    op1=mybir.AluOpType.mult,
                )

                xT = xt_pool.tile([P, KC, P], f32r, tag="xT", name="xT")
                for g in range(4):
                    pt = pt_psum.tile([P, 4, P], f32, tag="pt", name="pt")
                    for j in range(4):
                        c = 4 * g + j
                        nc.tensor.matmul(
                            out=pt[:, j, :],
                            lhsT=xt[:, bass.ts(c, P)],
                            rhs=ident,
                            is_transpose=True,
                            start=(j == 0),
                            stop=(j == 3),
                        )
                    nc.vector.tensor_copy(xT[:, bass.ts(g, 4), :], pt)
                return xT

            for i in range(NT):
                # software pipeline: tile i+1's transposes are emitted before
                # tile i's matmuls, so the PE fills its wait-for-DVE-copies
                # gap with useful transpose work.
                if i == 0:
                    if pending_xT is None:
                        pending_xT = prep(src, 0)
                xT = pending_xT
                if i + 1 < NT:
                    pending_xT = prep(src, i + 1)
                elif step + 1 < n_steps:
                    # cross-layer: tile 0 of the next step reads dst[0],
                    # which was written back at i == 0 of this step.
                    nxt_src = srcs[step + 1].rearrange("(n p) d -> n p d", p=P)
                    pending_xT = prep(nxt_src, 0)
                else:
                    pending_xT = None

                # --- matmul + bias + GELU ---
                yt = y_pool.tile([P, D], f32, tag="y")
                accs = [
                    acc_psum.tile([P, EW], f32, tag="acc", name="acc")
                    for _ in range(EC)
                ]
                for k in range(KC):
                    for e in range(EC):
                        mm = nc.tensor.matmul(
                            out=accs[e],
                            lhsT=xT[:, k, :],
                            rhs=wts[k][:, bass.ts(e, EW)],
                            start=(k == 0),
                            stop=(k == KC - 1),
                        ).ins
                        if i == 0 and e == 0:
                            add_dep_helper(
                                mm, wt_probes[k], False, "order after probe"
                            )
                for e in range(EC):
                    nc.vector.tensor_add(
                        yt[:, bass.ts(e, EW)],
                        accs[e],
                        brep[:, bass.ts(e, EW)],
                    )
                    nc.scalar.activation(
                        out=yt[:, bass.ts(e, EW)],
                        in_=yt[:, bass.ts(e, EW)],
                        func=mybir.ActivationFunctionType.Gelu_apprx_tanh,
                    )
                nc.sync.dma_start(out=dst[i], in_=yt)

    _split_matmul_waits(nc)


def _split_matmul_waits(nc):
    """Walrus encodes fp32/fp32r/transpose matmuls as self-loading LW-struct
    instructions, which accept at most ONE sync-wait command. Tile's wait
    assignment can attach several. Hoist all but one wait of each matmult onto
    standalone EventSemaphore (sequencer) instructions inserted right before
    it on the same engine — semantically identical, codegen-legal."""
    import concourse.mybir as mybir

    skip = ("InstEventSemaphore",)
    n_split = 0
    for fn in nc.m.functions:
        for bb in fn.blocks:
            insts = bb.instructions
            i = 0
            while i < len(insts):
                inst = insts[i]
                if type(inst).__name__ not in skip:
                    si = inst.sync_info
                    waits = list(si.on_wait) if (si and si.on_wait) else []
                    if len(waits) > 1:
                        for j, w in enumerate(waits[:-1]):
                            ev = mybir.InstEventSemaphore(
                                name=f"{inst.name}-hw{j}",
                                engine=inst.engine,
                                sync_info=mybir.SyncInfo(
                                    on_wait=[w], on_update=[]
                                ),
                            )
                            nc.register_instruction(ev, overwrite=True)
                            insts.insert(i, ev)
                            i += 1
                        si.on_wait = [waits[-1]]
                        n_split += 1
                i += 1
    return n_split


_CACHE = {}


def _get_nc():
    if "nc" not in _CACHE:
        import concourse.bass as bass

        nc = bass.Bass("TRN2", target_bir_lowering=False)
        build(nc)
        _CACHE["nc"] = nc
    return _CACHE["nc"]


def _prep_host(x, W, b, ln_w, ln_b):
    """Fold LN affine into weights; pre-transpose W to [L, D_in, D_out]."""
    x = np.ascontiguousarray(np.asarray(x, dtype=np.float32))
    W = np.asarray(W, dtype=np.float32)
    b = np.asarray(b, dtype=np.float32)
    ln_w = np.asarray(ln_w, dtype=np.float32)
    ln_b = np.asarray(ln_b, dtype=np.float32)

    Wf = W * ln_w[:, None, :]  # scale columns (input dim)
    bf = b + np.einsum("led,ld->le", W, ln_b)
    WT = np.ascontiguousarray(Wf.transpose(0, 2, 1))  # [L, D(in), E(out)]
    return x, WT, bf


def _enable_ldw_opt():
    """walrus's LDWEIGHTS-reload elision is disabled by the concourse driver
    flags; enabling it saves ~270us here (stationary operand reused across 4
    consecutive matmuls). Verified bit-identical output on hardware."""
    from concourse import bass_utils

    if getattr(bass_utils, "_ldw_opt_patched", False):
        return
    orig = bass_utils.run_command

    def patched(argv, **kw):
        argv = [
            "--enable-ldw-opt=true" if a == "--enable-ldw-opt=false" else a
            for a in argv
        ]
        return orig(argv, **kw)

    bass_utils.run_command = patched
    bass_utils._ldw_opt_patched = True


def run(x, W, b, ln_w, ln_b, trace=False):
    from concourse import bass_utils

    _enable_ldw_opt()

    x, WT, bf = _prep_host(x, W, b, ln_w, ln_b)
    nc = _get_nc()
    in_maps = [{"x": x[i], "wt": WT, "b": bf} for i in range(B)]
    res = bass_utils.run_bass_kernel_spmd(
        nc, in_maps, core_ids=list(range(B)), trace=trace
    )
    out = np.stack([res.results[i]["y"] for i in range(B)])
    return out.reshape(B, S, D), res


def kernel(x, W, b, ln_w, ln_b):
    out, _ = run(x, W, b, ln_w, ln_b)
    return out



# revision 6
# speedup vs baseline: 1.1669x; 1.1669x over previous
"""Trainium2 Bass kernel for nn_AttnMLP: 4x (LayerNorm -> Linear(2048,2048) -> tanh-GELU).

Sharding: data-parallel, batch dim (8 batch elements) across 8 NeuronCores.

v2 design (vs the fp32r baseline):
  - bf16 activations+weights (matmul rate is 1 cyc/row for both fp32r and
    bf16 on TRN2, but bf16 halves DMA traffic and SBUF footprint).
  - Activations SBUF-resident across all 4 layers: 16 slots of [128, 2048]
    bf16, written in place by the epilogue (no DRAM round-trips).
  - x-transposes moved off the PE onto the DMA XBAR: one dma_start_transpose
    per token tile (SBUF->SBUF, bf16) produces xT [128, 16, 128] directly.
    PE stream is pure matmul (~874us/pass floor at 2.4GHz).
  - Weights streamed per layer in bf16, double-buffered (prefetched one
    layer ahead, one chunk per token tile, on the Act HWDGE queue; the
    latency-critical transposes have the SP HWDGE queue to themselves).
  - Bias broadcast [128, 2048] precomputed on host (brep input).
  - LN affine (ln_w, ln_b) folded into W and b on the host:
      W' = W * ln_w[None, :],  b' = b + W @ ln_b
  - Output written bf16, upcast to fp32 on host.

Per-core per-tile pipeline (software-pipelined `lookahead` tiles ahead):
  prep(i):  DVE bn_stats/bn_aggr -> rstd (ScalarE sqrt, DVE recip);
            DVE in-place normalize of resident slot; SP dma_start_transpose
            slot -> xT[128, 16, 128]
  matmul(i): for k in 16: lhsT=xT[:,k,:] (stationary, LDW elided over e);
             for e in 4: PSUM[e] += xT_k.T @ W_k[:, 512e] (bf16, 512 rows)
  epilogue(i): DVE tensor_add (PSUM + brep -> slot, bf16), ScalarE
             Gelu_apprx_tanh in place; last layer: DMA slot -> y.
"""

import sys

sys.path.insert(0, "/opt/trn_rl_repo")

import numpy as np

N_LAYERS = 4
D = 2048  # embedding dim
B = 8  # batch (one element per core)
S = 2048  # sequence length
T = S  # tokens per core
P = 128  # partitions
KC = D // P  # 16 contraction chunks
EC = 4  # output-feature chunks (PSUM banks per tile)
EW = D // EC  # 512 output features per chunk
LN_EPS = 1e-5


def build(nc, T_tokens=T, n_layers=N_LAYERS, repeat=1, lookahead=1,
          transpose_mode="dma"):
    """Emit the kernel IR into `nc`.

    repeat > 1 re-runs the whole n_layers stack that many times (reusing the
    same weights) — a timing-only amplifier for slope measurements."""
    import concourse.bass as bass
    import concourse.mybir as mybir
    import concourse.tile as tile
    from contextlib import ExitStack

    f32 = mybir.dt.float32
    bf16 = mybir.dt.bfloat16
    NT = T_tokens // P  # token tiles
    n_steps = n_layers * repeat

    x_d = nc.dram_tensor("x", [T_tokens, D], bf16, kind="ExternalInput")
    wt_d = nc.dram_tensor("wt", [n_layers, D, D], bf16, kind="ExternalInput")
    br_d = nc.dram_tensor("brep", [n_layers, P, D], bf16, kind="ExternalInput")
    y_d = nc.dram_tensor("y", [T_tokens, D], bf16, kind="ExternalOutput")

    x_v = x_d.rearrange("(n p) d -> n p d", p=P)
    y_v = y_d.rearrange("(n p) d -> n p d", p=P)
    wt_v = wt_d.rearrange("l (kc p) e -> l kc p e", p=P)  # [L, 16, 128, 2048]

    with tile.TileContext(nc) as tc, ExitStack() as ctx:
        singles = ctx.enter_context(tc.tile_pool(name="singles", bufs=1))
        xres_pool = ctx.enter_context(tc.tile_pool(name="xres", bufs=1))
        # chunk 0 single-buffered to fit SBUF; its next-layer DMA is issued
        # at tile 15 (after its last read of the layer) so the WAR wait is
        # already satisfied and the transfer overlaps tile 15's matmuls.
        wt_pool1 = ctx.enter_context(tc.tile_pool(name="wt1", bufs=1))
        wt_pool = ctx.enter_context(tc.tile_pool(name="wt", bufs=2))
        brep_pool = ctx.enter_context(tc.tile_pool(name="brep", bufs=2))
        xt_pool = ctx.enter_context(tc.tile_pool(name="xt", bufs=lookahead + 1))
        st_pool = ctx.enter_context(tc.tile_pool(name="st", bufs=lookahead + 2))
        acc_psum = ctx.enter_context(tc.tile_pool(name="accp", bufs=2, space="PSUM"))

        eps_t = singles.tile([P, 1], f32)
        nc.vector.memset(eps_t, LN_EPS)

        xres = [xres_pool.tile([P, D], bf16, tag=f"xres{i}", name=f"xres{i}")
                for i in range(NT)]

        def alloc_wts():
            return [
                (wt_pool1 if k == 0 else wt_pool).tile(
                    [P, D], bf16, tag=f"wt{k}", name=f"wt{k}"
                )
                for k in range(KC)
            ]

        # --- initial loads (Act HWDGE queue) ---
        n_first = min(lookahead + 2, NT)
        for i in range(n_first):
            nc.scalar.dma_start(out=xres[i], in_=x_v[i])
        wts_cur = alloc_wts()
        brep_cur = brep_pool.tile([P, D], bf16, tag="brep", name="brep")
        nc.scalar.dma_start(out=brep_cur, in_=br_d[0])
        for k in range(KC):
            nc.scalar.dma_start(out=wts_cur[k], in_=wt_v[0, k])
            i = n_first + k
            if i < NT:
                nc.scalar.dma_start(out=xres[i], in_=x_v[i])

        def prep(i):
            """LN stats + in-place normalize + transpose for token tile i of
            the current layer; returns the ready-to-contract xT tile."""
            slot = xres[i]
            stats = st_pool.tile([P, 4, 6], f32, tag="bnst", name="stats")
            for g in range(4):
                nc.vector.bn_stats(out=stats[:, g, :], in_=slot[:, bass.ts(g, 512)])
            mv = st_pool.tile([P, 2], f32, tag="mv", name="mv")
            nc.vector.bn_aggr(out=mv, in_=stats)
            rstd = st_pool.tile([P, 1], f32, tag="rstd", name="rstd")
            nc.scalar.activation(
                out=rstd,
                in_=mv[:, 1:2],
                func=mybir.ActivationFunctionType.Sqrt,
                bias=eps_t,
                scale=1.0,
            )
            nc.vector.reciprocal(out=rstd, in_=rstd)
            nc.vector.tensor_scalar(
                out=slot,
                in0=slot,
                scalar1=mv[:, 0:1],
                scalar2=rstd,
                op0=mybir.AluOpType.subtract,
                op1=mybir.AluOpType.mult,
            )
            xT = xt_pool.tile([P, KC, P], bf16, tag="xT", name="xT")
            nc.sync.dma_start_transpose(out=xT, in_=slot)
            return xT

        total = n_steps * NT
        wts_by_step = [wts_cur]
        brep_by_step = [brep_cur]
        from collections import deque

        q = deque()
        for j in range(min(lookahead, total)):
            q.append(prep(j % NT))

        for j in range(total):
            step, i = divmod(j, NT)
            l = step % n_layers
            if j + lookahead < total:
                q.append(prep((j + lookahead) % NT))
            xT = q.popleft()
            wts = wts_by_step[step]
            brep = brep_by_step[step]

            # prefetch layer step+1: chunks 1..15 spread round-robin over the
            # tiles; the single-buffered chunk 0 last (tile NT-1), after its
            # final read of the layer so the WAR wait is already satisfied.
            if step + 1 < n_steps:
                nl = (step + 1) % n_layers
                if i == 0:
                    wts_by_step.append(alloc_wts())
                    bnx = brep_pool.tile([P, D], bf16, tag="brep", name="brep")
                    brep_by_step.append(bnx)
                    nc.scalar.dma_start(out=bnx, in_=br_d[nl])
                for c in range(1, KC):
                    if c % NT == i:
                        nc.scalar.dma_start(
                            out=wts_by_step[step + 1][c], in_=wt_v[nl, c]
                        )
                if i == NT - 1:
                    nc.scalar.dma_start(
                        out=wts_by_step[step + 1][0], in_=wt_v[nl, 0]
                    )

            # --- matmuls ---
            accs = [
                acc_psum.tile([P, EW], f32, tag=f"acc{e}", name=f"acc{e}")
                for e in range(EC)
            ]
            for k in range(KC):
                for e in range(EC):
                    nc.tensor.matmul(
                        out=accs[e],
                        lhsT=xT[:, k, :],
                        rhs=wts[k][:, bass.ts(e, EW)],
                        start=(k == 0),
                        stop=(k == KC - 1),
                    )

            # --- epilogue: bias add + GELU, in place into the resident slot
            slot = xres[i]
            for e in range(EC):
                nc.vector.tensor_add(
                    slot[:, bass.ts(e, EW)], accs[e], brep[:, bass.ts(e, EW)]
                )
            nc.scalar.activation(
                out=slot,
                in_=slot,
                func=mybir.ActivationFunctionType.Gelu_apprx_tanh,
            )
            if l == n_layers - 1:
                nc.scalar.dma_start(out=y_v[i], in_=slot)

    _elide_ldweights(nc)
    _split_matmul_waits(nc)


def _elide_ldweights(nc):
    """The tile finalize pass splits 16-bit matmuls into explicit
    InstLdweights + InstMatmult pairs, one load per matmul. Consecutive
    matmuls here share the stationary operand (lhsT reused across the 4
    e-chunks), so 3 of every 4 loads are redundant — the PE array already
    holds the weights. Drop them (keeping any sync as an EventSemaphore);
    saves 128 load-rows per 512 matmul rows (~20% PE time)."""
    import concourse.mybir as mybir

    n = 0
    for fn in nc.m.functions:
        for bb in fn.blocks:
            insts = bb.instructions
            last_key = None
            i = 0
            while i < len(insts):
                inst = insts[i]
                if getattr(inst, "engine", None) == mybir.EngineType.PE:
                    tn = type(inst).__name__
                    if tn == "InstLdweights":
                        ap = inst.ins[0]
                        key = (
                            str(getattr(ap, "memref", None)),
                            getattr(ap, "offset", None),
                            str(getattr(ap, "ap", None)),
                            str(getattr(ap, "dtype", None)),
                            bool(inst.is_transpose),
                            str(inst.perf_mode),
                        )
                        if key == last_key:
                            si = inst.sync_info
                            if si and (si.on_wait or si.on_update):
                                ev = mybir.InstEventSemaphore(
                                    name=f"{inst.name}-ldwe",
                                    engine=inst.engine,
                                    sync_info=si,
                                )
                                nc.register_instruction(ev, overwrite=True)
                                insts[i] = ev
                                i += 1
                            else:
                                del insts[i]
                            n += 1
                            continue
                        last_key = key
                    elif tn in ("InstMatmult", "InstEventSemaphore"):
                        pass
                    else:
                        last_key = None
                i += 1
    return n


def _split_matmul_waits(nc):
    """Walrus encodes fp32/fp32r/transpose matmuls as self-loading LW-struct
    instructions, which accept at most ONE sync-wait command. Tile's wait
    assignment can attach several. Hoist all but one wait of each matmult onto
    standalone EventSemaphore (sequencer) instructions inserted right before
    it on the same engine — semantically identical, codegen-legal."""
    import concourse.mybir as mybir

    skip = ("InstEventSemaphore",)
    n_split = 0
    for fn in nc.m.functions:
        for bb in fn.blocks:
            insts = bb.instructions
            i = 0
            while i < len(insts):
                inst = insts[i]
                if type(inst).__name__ not in skip:
                    si = inst.sync_info
                    waits = list(si.on_wait) if (si and si.on_wait) else []
                    if len(waits) > 1:
                        for j, w in enumerate(waits[:-1]):
                            ev = mybir.InstEventSemaphore(
                                name=f"{inst.name}-hw{j}",
                                engine=inst.engine,
                                sync_info=mybir.SyncInfo(
                                    on_wait=[w], on_update=[]
                                ),
                            )
                            nc.register_instruction(ev, overwrite=True)
                            insts.insert(i, ev)
                            i += 1
                        si.on_wait = [waits[-1]]
                        n_split += 1
                i += 1
    return n_split


_CACHE = {}


def _get_nc():
    if "nc" not in _CACHE:
        import concourse.bass as bass

        nc = bass.Bass("TRN2", target_bir_lowering=False)
        build(nc)
        _CACHE["nc"] = nc
    return _CACHE["nc"]


def _prep_host(x, W, b, ln_w, ln_b):
    """Fold LN affine into weights; pre-transpose W to [L, D_in, D_out];
    cast everything to bf16 and build the replicated bias input."""
    import ml_dtypes

    bf = ml_dtypes.bfloat16
    x = np.asarray(x, dtype=np.float32)
    W = np.asarray(W, dtype=np.float32)
    b = np.asarray(b, dtype=np.float32)
    ln_w = np.asarray(ln_w, dtype=np.float32)
    ln_b = np.asarray(ln_b, dtype=np.float32)

    Wf = W * ln_w[:, None, :]  # scale columns (input dim)
    bfold = b + np.einsum("led,ld->le", W, ln_b)
    WT = np.ascontiguousarray(Wf.transpose(0, 2, 1)).astype(bf)  # [L, Din, E]
    brep = np.ascontiguousarray(
        np.broadcast_to(bfold[:, None, :], (bfold.shape[0], P, D))
    ).astype(bf)
    xb = np.ascontiguousarray(x).astype(bf)
    return xb, WT, brep


def make_in_maps(inputs):
    xb, WT, brep = _prep_host(**inputs)
    return [{"x": xb[i], "wt": WT, "brep": brep} for i in range(B)]


def _enable_ldw_opt():
    """walrus's LDWEIGHTS-reload elision is disabled by the concourse driver
    flags; enabling it saves the stationary-operand reload across the 4
    consecutive matmuls that share an lhsT."""
    from concourse import bass_utils

    if getattr(bass_utils, "_ldw_opt_patched", False):
        return
    orig = bass_utils.run_command

    def patched(argv, **kw):
        argv = [
            "--enable-ldw-opt=true" if a == "--enable-ldw-opt=false" else a
            for a in argv
        ]
        return orig(argv, **kw)

    bass_utils.run_command = patched
    bass_utils._ldw_opt_patched = True


def run(x, W, b, ln_w, ln_b, trace=False):
    from concourse import bass_utils

    in_maps = make_in_maps(dict(x=x, W=W, b=b, ln_w=ln_w, ln_b=ln_b))
    nc = _get_nc()
    res = bass_utils.run_bass_kernel_spmd(
        nc, in_maps, core_ids=list(range(B)), trace=trace
    )
    out = np.stack(
        [np.asarray(res.results[i]["y"], dtype=np.float32) for i in range(B)]
    )
    return out.reshape(B, S, D), res


def kernel(x, W, b, ln_w, ln_b):
    out, _ = run(x, W, b, ln_w, ln_b)
    return out
